# revision 22
# baseline (speedup 1.0000x reference)
"""Trainium2 Bass kernel for nn_EnhancedGIN (3x GINConv + global mean pool + head).

Strategy (8 NeuronCores, SPMD — one program, per-core data):
  * Nodes/edges sharded by destination-node range (128-aligned blocks).
  * Per layer: bulk dma_gather of source-node features (table in HBM; 4
    src-range buckets to fit int16 indices), scatter-sum via one-hot
    "segment matmul" into PSUM per dst block, feature-major MLP with BN
    folded into weights, GELU via ACT.
  * Node features AllGathered (bf16) between layers.
  * Global mean pool via one-hot matmul + AllReduce; small replicated head.
Host side only shards/sorts/pads indices and folds BN constants.
"""

import os
import sys

for _p in ("/opt/trn_rl_repo", "/root/.axon_site/_ro/trn_rl_repo"):
    if os.path.isdir(_p) and _p not in sys.path:
        sys.path.insert(0, _p)

import numpy as np
import ml_dtypes

import concourse.bacc as bacc
import concourse.mybir as mybir
import concourse.tile as tile
from concourse import bass_utils, library_config
from concourse.bass import IndirectOffsetOnAxis

P = 128
NCORES = 8
NBUCK = 4
BN_EPS = 1e-5
LN_EPS = 1e-5
BF16 = mybir.dt.bfloat16
F32 = mybir.dt.float32
I16 = mybir.dt.int16
I32 = mybir.dt.int32
nbf = ml_dtypes.bfloat16

GELU_FN = mybir.ActivationFunctionType.Gelu


def _fold_bn(g, b, m, v):
    s = g / np.sqrt(v + BN_EPS)
    return s.astype(np.float32), (b - m * s).astype(np.float32)


def _prepare(inputs, chunk_blocks=3):
    """Host-side data prep: shard + sort edges, pad, fold BN into weights."""
    x = np.asarray(inputs["x"], np.float32)
    ei = np.asarray(inputs["edge_index"]).astype(np.int64)
    batch = np.asarray(inputs["batch"]).astype(np.int64)
    src, dst = ei[0], ei[1]
    N, IN_DIM = x.shape
    E = src.shape[0]
    HID = np.asarray(inputs["c0_w2"]).shape[0]
    LAT = np.asarray(inputs["fc2_w"]).shape[1]
    G = int(batch.max()) + 1 if batch.size else 1
    GP = -(-max(G, 1) // P) * P
    NGT = GP // P
    CB = chunk_blocks

    NBLK = -(-N // (NCORES * P))          # dst blocks per core
    SH = NBLK * P                          # node rows per core (padded)
    NROWS = SH * NCORES                    # table rows (padded)
    assert NROWS % NBUCK == 0
    BUCKET = NROWS // NBUCK
    assert BUCKET <= (1 << 15) - 1, BUCKET

    # ---- edge structure: sort by (dst block, src bucket, src) ----
    blk = dst // P                         # global dst block
    bkt = src // BUCKET
    order = np.lexsort((src, bkt, blk))
    ssrc = src[order]
    sdst = dst[order]
    nblk_total = NBLK * NCORES
    key = blk[order] * NBUCK + bkt[order]
    counts = np.bincount(key, minlength=nblk_total * NBUCK).reshape(
        nblk_total, NBUCK)
    starts = np.zeros(nblk_total * NBUCK + 1, np.int64)
    np.cumsum(counts.reshape(-1), out=starts[1:])

    # per (chunk, bucket, block-in-chunk) tile count: max over cores
    nchunk = -(-NBLK // CB)
    tiles_cb = -(-counts // P)             # [nblk_total, NBUCK] ceil tiles
    chunks = []
    for ch in range(nchunk):
        b0 = ch * CB
        cb = min(CB, NBLK - b0)
        segs = []
        for k in range(NBUCK):
            row = []
            for bi in range(cb):
                t = 0
                for c in range(NCORES):
                    t = max(t, int(tiles_cb[c * NBLK + b0 + bi, k]))
                if k == 0:
                    t = max(t, 1)          # guarantee a start tile per block
                row.append(t)
            segs.append(row)
        chunks.append(dict(cb=cb, segs=segs))

    NT = sum(sum(sum(s) for s in m["segs"]) for m in chunks)  # total tiles

    idx16_all = np.zeros((NCORES, NT * 8, 16), np.int16)   # wrapped cols
    dstv_all = np.full((NCORES, NT, P), -1.0, np.float32)
    for c in range(NCORES):
        tt = 0
        for ch, m in enumerate(chunks):
            b0, cb = ch * CB, m["cb"]
            for k in range(NBUCK):
                for bi in range(cb):
                    T = m["segs"][k][bi]
                    gb = c * NBLK + b0 + bi
                    s0 = starts[gb * NBUCK + k]
                    n = counts[gb, k]
                    slots = T * P
                    seg_i = np.zeros(slots, np.int16)
                    seg_d = np.full(slots, -1.0, np.float32)
                    seg_i[:n] = (ssrc[s0:s0 + n] - k * BUCKET).astype(np.int16)
                    seg_d[:n] = (sdst[s0:s0 + n] - gb * P).astype(np.float32)
                    idx16_all[c, tt * 8:(tt + T) * 8] = seg_i.reshape(-1, 16)
                    dstv_all[c, tt:tt + T] = seg_d.reshape(T, P)
                    tt += T
        assert tt == NT
    # idx wrapped: value i at partition i%16, column i//16, replicated x8
    idx16 = np.ascontiguousarray(
        np.tile(idx16_all.transpose(0, 2, 1), (1, 8, 1)))   # [NCORES,128,NT*8]
    dstv_pt = np.ascontiguousarray(dstv_all.transpose(0, 2, 1))  # [NC,P,NT]

    # ---- node-side per-core data ----
    deg = np.bincount(dst, minlength=N).astype(np.float32)
    eps0 = float(np.asarray(inputs["eps0"]))
    epsR = [float(v) for v in np.asarray(inputs["epsR"])]

    xpad = np.zeros((NROWS, IN_DIM), np.float32)
    xpad[:N] = x

    degpad = np.zeros(NROWS, np.float32)
    degpad[:N] = deg + 1.0 + eps0

    xaugs = []
    for c in range(NCORES):
        xa = np.zeros((IN_DIM + 1, SH), np.float32)
        xa[:IN_DIM] = xpad[c * SH:(c + 1) * SH].T
        xa[IN_DIM] = degpad[c * SH:(c + 1) * SH]
        xaugs.append(xa)

    # ---- pooling data ----
    bpad = np.full(NROWS, -1, np.int64)
    bpad[:N] = batch
    spools, gidxs = [], []
    for c in range(NCORES):
        bc = bpad[c * SH:(c + 1) * SH]
        real = bc >= 0
        g0 = int(bc[real].min()) if real.any() else 0
        w = int(bc[real].max()) - g0 + 1 if real.any() else 1
        assert w <= P, f"core {c} graph window {w} > {P}"
        sp = np.where(real, bc - g0, -1).astype(np.float32)
        spools.append(np.ascontiguousarray(sp.reshape(NBLK, P).T))
        gidxs.append((g0 + np.arange(P)).astype(np.int32)[:, None])

    cnt = np.bincount(batch, minlength=GP).astype(np.float32)
    invc = np.zeros(GP, np.float32)
    invc[:G] = 1.0 / np.maximum(cnt[:G], 1.0)
    invc_pt = np.ascontiguousarray(invc.reshape(NGT, P).T)

    # ---- folded weights ----
    sbn, cbn = _fold_bn(np.asarray(inputs["in_g"], np.float32),
                        np.asarray(inputs["in_b"], np.float32),
                        np.asarray(inputs["in_m"], np.float32),
                        np.asarray(inputs["in_v"], np.float32))
    s1, c1 = _fold_bn(np.asarray(inputs["c0_g"], np.float32),
                      np.asarray(inputs["c0_bb"], np.float32),
                      np.asarray(inputs["c0_m"], np.float32),
                      np.asarray(inputs["c0_v"], np.float32))
    w1f = np.asarray(inputs["c0_w1"], np.float32) * s1[None, :]
    b1f = np.asarray(inputs["c0_b1"], np.float32) * s1 + c1
    w1aug0 = np.vstack([sbn[:, None] * w1f, (cbn @ w1f)[None, :]])
    s2_0, c2_0 = _fold_bn(np.asarray(inputs["bn0_g"], np.float32),
                          np.asarray(inputs["bn0_b"], np.float32),
                          np.asarray(inputs["bn0_m"], np.float32),
                          np.asarray(inputs["bn0_v"], np.float32))
    b2f_0 = np.asarray(inputs["c0_b2"], np.float32) * s2_0 + c2_0

    layers = [(w1aug0.astype(np.float32), b1f[:, None],
               np.asarray(inputs["c0_w2"], np.float32),
               s2_0[:, None], b2f_0[:, None], eps0)]
    for l in range(2):
        s1l, c1l = _fold_bn(np.asarray(inputs["cR_g"][l], np.float32),
                            np.asarray(inputs["cR_bb"][l], np.float32),
                            np.asarray(inputs["cR_m"][l], np.float32),
                            np.asarray(inputs["cR_v"][l], np.float32))
        w1l = np.asarray(inputs["cR_w1"][l], np.float32) * s1l[None, :]
        b1l = np.asarray(inputs["cR_b1"][l], np.float32) * s1l + c1l
        s2l, c2l = _fold_bn(np.asarray(inputs["bnR_g"][l], np.float32),
                            np.asarray(inputs["bnR_b"][l], np.float32),
                            np.asarray(inputs["bnR_m"][l], np.float32),
                            np.asarray(inputs["bnR_v"][l], np.float32))
        b2l = np.asarray(inputs["cR_b2"][l], np.float32) * s2l + c2l
        layers.append((w1l, b1l[:, None],
                       np.asarray(inputs["cR_w2"][l], np.float32),
                       s2l[:, None], b2l[:, None], epsR[l]))

    iota = np.tile(np.arange(P, dtype=np.float32)[None, :], (P, 1))
    ident = np.eye(P, dtype=np.float32)

    shared = {
        "xtab": xpad,                       # f32 table for layer-0 gather
        "invc": invc_pt,
        "iotab": iota.astype(nbf),
        "iotaf": iota.astype(np.float32),
        "ident": ident,
        "fc1w": np.asarray(inputs["fc1_w"], np.float32),
        "fc1br": np.tile(np.asarray(inputs["fc1_b"], np.float32)[None, :], (P, 1)),
        "lngr": np.tile(np.asarray(inputs["ln_g"], np.float32)[None, :], (P, 1)),
        "lnbr": np.tile(np.asarray(inputs["ln_b"], np.float32)[None, :], (P, 1)),
        "fc2w": np.asarray(inputs["fc2_w"], np.float32),
        "fc2br": np.tile(np.asarray(inputs["fc2_b"], np.float32)[None, :], (P, 1)),
    }
    for li, (w1, b1c, w2, s2c, b2c, _e) in enumerate(layers):
        shared[f"w1_{li}"] = w1
        shared[f"b1c_{li}"] = b1c
        shared[f"w2_{li}"] = w2
        shared[f"sc2_{li}"] = s2c
        shared[f"bc2_{li}"] = b2c

    in_maps = []
    for c in range(NCORES):
        m = dict(shared)
        m["idx"] = idx16[c]
        m["dstv"] = dstv_pt[c].reshape(P, NT)
        m["xaug"] = xaugs[c]
        m["spool"] = spools[c]
        m["gidx"] = gidxs[c]
        in_maps.append(m)

    struct = dict(N=N, E=E, G=G, GP=GP, NGT=NGT, IN_DIM=IN_DIM, HID=HID,
                  LAT=LAT, NBLK=NBLK, SH=SH, NROWS=NROWS, BUCKET=BUCKET,
                  NT=NT, CB=CB, chunks=chunks, eps=[la[5] for la in layers])
    return struct, in_maps


def _build(st, debug_outs=False):
    """Build the SPMD Bass program for one core (identical across cores)."""
    N, IN_DIM, HID, LAT = st["N"], st["IN_DIM"], st["HID"], st["LAT"]
    NBLK, SH, NROWS = st["NBLK"], st["SH"], st["NROWS"]
    NT, GP, NGT = st["NT"], st["GP"], st["NGT"]
    BUCKET, CB, chunks = st["BUCKET"], st["CB"], st["chunks"]
    eps = st["eps"]
    maxtile = max(sum(sum(s) for s in m["segs"]) for m in chunks)

    nc = bacc.Bacc("TRN2", target_bir_lowering=False, debug=False,
                   num_devices=NCORES)

    xtab = nc.dram_tensor("xtab", [NROWS, IN_DIM], F32, kind="ExternalInput")
    idx_d = nc.dram_tensor("idx", [P, NT * 8], I16, kind="ExternalInput")
    dstv_d = nc.dram_tensor("dstv", [P, NT], F32, kind="ExternalInput")
    xaug_d = nc.dram_tensor("xaug", [IN_DIM + 1, SH], F32, kind="ExternalInput")
    spool_d = nc.dram_tensor("spool", [P, NBLK], F32, kind="ExternalInput")
    gidx_d = nc.dram_tensor("gidx", [P, 1], I32, kind="ExternalInput")
    invc_d = nc.dram_tensor("invc", [P, NGT], F32, kind="ExternalInput")
    iotab_d = nc.dram_tensor("iotab", [P, P], BF16, kind="ExternalInput")
    iotaf_d = nc.dram_tensor("iotaf", [P, P], F32, kind="ExternalInput")
    ident_d = nc.dram_tensor("ident", [P, P], F32, kind="ExternalInput")
    wd = {}
    for li in range(3):
        K = IN_DIM + 1 if li == 0 else HID
        wd[f"w1_{li}"] = nc.dram_tensor(f"w1_{li}", [K, HID], F32, kind="ExternalInput")
        wd[f"b1c_{li}"] = nc.dram_tensor(f"b1c_{li}", [HID, 1], F32, kind="ExternalInput")
        wd[f"w2_{li}"] = nc.dram_tensor(f"w2_{li}", [HID, HID], F32, kind="ExternalInput")
        wd[f"sc2_{li}"] = nc.dram_tensor(f"sc2_{li}", [HID, 1], F32, kind="ExternalInput")
        wd[f"bc2_{li}"] = nc.dram_tensor(f"bc2_{li}", [HID, 1], F32, kind="ExternalInput")
    fc1w_d = nc.dram_tensor("fc1w", [HID, HID], F32, kind="ExternalInput")
    fc1br_d = nc.dram_tensor("fc1br", [P, HID], F32, kind="ExternalInput")
    lngr_d = nc.dram_tensor("lngr", [P, HID], F32, kind="ExternalInput")
    lnbr_d = nc.dram_tensor("lnbr", [P, HID], F32, kind="ExternalInput")
    fc2w_d = nc.dram_tensor("fc2w", [HID, LAT], F32, kind="ExternalInput")
    fc2br_d = nc.dram_tensor("fc2br", [P, LAT], F32, kind="ExternalInput")
    out_d = nc.dram_tensor("out", [GP, LAT], F32, kind="ExternalOutput")
    if debug_outs:
        dbg_h0 = nc.dram_tensor("dbg_h0", [HID, SH], F32, kind="ExternalOutput")
        dbg_tab0 = nc.dram_tensor("dbg_tab0", [NROWS, HID], BF16,
                                  kind="ExternalOutput")
        dbg_pool = nc.dram_tensor("dbg_pool", [GP, HID], F32,
                                  kind="ExternalOutput")
        dbg_agg0 = nc.dram_tensor("dbg_agg0", [IN_DIM, P], F32,
                                  kind="ExternalOutput")

    groups = [list(range(NCORES))]
    add = mybir.AluOpType.add
    mult = mybir.AluOpType.mult
    iseq = mybir.AluOpType.is_equal

    with tile.TileContext(nc) as tc:
        import contextlib
        ctx = contextlib.ExitStack()
        with ctx:
            dram = ctx.enter_context(tc.tile_pool(name="dram", bufs=1, space="DRAM"))
            persist = ctx.enter_context(tc.tile_pool(name="persist", bufs=1))
            gpool = ctx.enter_context(tc.tile_pool(name="gpool", bufs=2))
            idxp = ctx.enter_context(tc.tile_pool(name="idxp", bufs=2))
            sgen_p = ctx.enter_context(tc.tile_pool(name="sgen", bufs=4))
            psA_p = ctx.enter_context(tc.tile_pool(name="psA", bufs=CB, space="PSUM"))
            psZ_p = ctx.enter_context(tc.tile_pool(name="psZ", bufs=2, space="PSUM"))
            psT_p = ctx.enter_context(tc.tile_pool(name="psT", bufs=2, space="PSUM"))
            psPool_p = ctx.enter_context(tc.tile_pool(name="psPool", bufs=1, space="PSUM"))
            small = ctx.enter_context(tc.tile_pool(name="small", bufs=3))
            stg_p = ctx.enter_context(tc.tile_pool(name="stg", bufs=3))
            head_p = ctx.enter_context(tc.tile_pool(name="head", bufs=2))
            pt_p = ctx.enter_context(tc.tile_pool(name="ptp", bufs=1))

            tabin = dram.tile([SH, HID], BF16, name="tabin")
            tabouts = [dram.tile([NROWS, HID], BF16, name=f"tabout{i}",
                                 addr_space="Shared") for i in range(2)]
            pools_t = dram.tile([GP, HID], F32, name="pools")
            poolr_t = dram.tile([GP, HID], F32, name="poolr", addr_space="Shared")

            nc.gpsimd.load_library(library_config.mlp)

            dstv_sb = persist.tile([P, NT], F32, name="dstv_sb")
            xaug_sb = persist.tile([IN_DIM + 1, SH], F32, name="xaug_sb")
            hT_sb = persist.tile([HID, SH], F32, name="hT_sb")
            spool_sb = persist.tile([P, NBLK], F32, name="spool_sb")
            gidx_sb = persist.tile([P, 1], I32, name="gidx_sb")
            invc_sb = persist.tile([P, NGT], F32, name="invc_sb")
            iotab_sb = persist.tile([P, P], BF16, name="iotab_sb")
            iotaf_sb = persist.tile([P, P], F32, name="iotaf_sb")
            ident_sb = persist.tile([P, P], F32, name="ident_sb")
            w_sb = {}
            for li in range(3):
                K = IN_DIM + 1 if li == 0 else HID
                w_sb[f"w1_{li}"] = persist.tile([K, HID], F32, name=f"w1s_{li}")
                w_sb[f"b1c_{li}"] = persist.tile([HID, 1], F32, name=f"b1cs_{li}")
                w_sb[f"w2_{li}"] = persist.tile([HID, HID], F32, name=f"w2s_{li}")
                w_sb[f"sc2_{li}"] = persist.tile([HID, 1], F32, name=f"sc2s_{li}")
                w_sb[f"bc2_{li}"] = persist.tile([HID, 1], F32, name=f"bc2s_{li}")
            fc1w_sb = persist.tile([HID, HID], F32, name="fc1w_sb")
            fc1br_sb = persist.tile([P, HID], F32, name="fc1br_sb")
            lngr_sb = persist.tile([P, HID], F32, name="lngr_sb")
            lnbr_sb = persist.tile([P, HID], F32, name="lnbr_sb")
            fc2w_sb = persist.tile([HID, LAT], F32, name="fc2w_sb")
            fc2br_sb = persist.tile([P, LAT], F32, name="fc2br_sb")
            pooledT_sb = persist.tile([HID, GP], F32, name="pooledT_sb")

            for sb, dr in [(dstv_sb, dstv_d), (xaug_sb, xaug_d),
                           (spool_sb, spool_d), (gidx_sb, gidx_d),
                           (invc_sb, invc_d), (iotab_sb, iotab_d),
                           (iotaf_sb, iotaf_d), (ident_sb, ident_d),
                           (fc1w_sb, fc1w_d), (fc1br_sb, fc1br_d),
                           (lngr_sb, lngr_d), (lnbr_sb, lnbr_d),
                           (fc2w_sb, fc2w_d), (fc2br_sb, fc2br_d)]:
                nc.sync.dma_start(sb[:], dr[:])
            for k, t in w_sb.items():
                nc.sync.dma_start(t[:], wd[k][:])

            psPool = psPool_p.tile([P, HID], F32, name="psPool")

            for li in range(3):
                F = IN_DIM if li == 0 else HID
                Kmm = IN_DIM + 1 if li == 0 else HID
                GDT = F32 if li == 0 else BF16
                iota_l = iotaf_sb if li == 0 else iotab_sb
                table = xtab if li == 0 else tabouts[li - 1]
                inbuf = xaug_sb if li == 0 else hT_sb
                epsl = eps[li]
                w1l = w_sb[f"w1_{li}"]
                b1l = w_sb[f"b1c_{li}"]
                w2l = w_sb[f"w2_{li}"]
                s2l = w_sb[f"sc2_{li}"]
                b2l = w_sb[f"bc2_{li}"]

                tt = 0          # global tile counter (matches dstv columns)
                for ch, m in enumerate(chunks):
                    b0, cb = ch * CB, m["cb"]
                    segs = m["segs"]
                    ctiles = sum(sum(s) for s in segs)
                    Gt = gpool.tile([P, maxtile * F], GDT, name="Gt",
                                    tag="Gt")
                    idxc = idxp.tile([P, maxtile * 8], I16, name="idxc",
                                     tag="idxc")
                    nc.sync.dma_start(idxc[:, :ctiles * 8],
                                      idx_d[:, tt * 8:(tt + ctiles) * 8])
                    # bulk gathers per (chunk, bucket); the SWDGE ring caps
                    # one call at ~1024 descriptors -> split to <=7 tiles
                    MGT = 7
                    soff = 0
                    for k in range(NBUCK):
                        seg_tiles = sum(segs[k])
                        done = 0
                        while done < seg_tiles:
                            stt = min(MGT, seg_tiles - done)
                            a = soff + done
                            nidx = stt * P
                            nc.gpsimd.dma_gather(
                                Gt[:, a * F:(a + stt) * F].rearrange(
                                    "p (t f) -> p t f", f=F),
                                table[k * BUCKET:(k + 1) * BUCKET, :],
                                idxc[:, a * 8:(a + stt) * 8],
                                nidx, nidx, F)
                            done += stt
                        soff += seg_tiles
                    # one-hot matmuls, accumulated per dst block
                    psAs = [psA_p.tile([F, P], F32, name=f"psA{bi}", tag="psA")
                            for bi in range(cb)]
                    klast = [max(k for k in range(NBUCK) if segs[k][bi] > 0)
                             for bi in range(cb)]
                    toff = 0
                    for k in range(NBUCK):
                        for bi in range(cb):
                            T = segs[k][bi]
                            if T == 0:
                                continue
                            St = sgen_p.tile([P, T * P], GDT, name="St",
                                             tag="St")
                            nc.vector.tensor_tensor(
                                out=St[:, :T * P].rearrange(
                                    "p (t j) -> p t j", j=P),
                                in0=dstv_sb[:, tt + toff:tt + toff + T, None
                                            ].to_broadcast([P, T, P]),
                                in1=iota_l[:, None, :].to_broadcast([P, T, P]),
                                op=iseq)
                            for t in range(T):
                                nc.tensor.matmul(
                                    psAs[bi][:, :],
                                    lhsT=Gt[:, (toff + t) * F:(toff + t + 1) * F],
                                    rhs=St[:, t * P:(t + 1) * P],
                                    start=(k == 0 and t == 0),
                                    stop=(k == klast[bi] and t == T - 1),
                                )
                            toff += T
                    tt += ctiles

                    # per-block epilogue: s = (1+eps)h + agg; MLP; transpose
                    for bi in range(cb):
                        blk = b0 + bi
                        cols = slice(blk * P, (blk + 1) * P)
                        psA = psAs[bi]
                        if debug_outs and li == 0 and blk == 0:
                            dbga = small.tile([IN_DIM, P], F32, name="dbga")
                            nc.vector.tensor_copy(dbga[:, :], psA[:, :])
                            nc.sync.dma_start(dbg_agg0[:, :], dbga[:, :])
                        nc.vector.tensor_scalar_mul(
                            inbuf[0:F, cols], inbuf[0:F, cols],
                            float(1.0 + epsl))
                        nc.vector.tensor_tensor(
                            out=inbuf[0:F, cols], in0=inbuf[0:F, cols],
                            in1=psA[:, :], op=add)
                        psZ = psZ_p.tile([HID, P], F32, name="psZ", tag="psZ")
                        nc.tensor.matmul(psZ[:, :], lhsT=w1l[:, :],
                                         rhs=inbuf[0:Kmm, cols],
                                         start=True, stop=True)
                        hmid = small.tile([HID, P], F32, name="hmid", tag="hmid")
                        nc.scalar.activation(hmid[:, :], psZ[:, :], GELU_FN,
                                             bias=b1l[:, :1], scale=1.0)
                        psZ2 = psZ_p.tile([HID, P], F32, name="psZ2", tag="psZ")
                        nc.tensor.matmul(psZ2[:, :], lhsT=w2l[:, :],
                                         rhs=hmid[:, :], start=True, stop=True)
                        nc.scalar.activation(hT_sb[:, cols], psZ2[:, :], GELU_FN,
                                             bias=b2l[:, :1], scale=s2l[:, :1])
                        psT = psT_p.tile([P, HID], F32, name="psT", tag="psT")
                        nc.tensor.transpose(psT[:, :], hT_sb[:, cols],
                                            ident_sb[:, :])
                        if li < 2:
                            stg = stg_p.tile([P, HID], BF16, name="stg",
                                             tag="stg")
                            nc.vector.tensor_copy(stg[:, :], psT[:, :])
                            nc.sync.dma_start(tabin[blk * P:(blk + 1) * P, :],
                                              stg[:, :])
                        else:
                            stgf = stg_p.tile([P, HID], F32, name="stgf",
                                              tag="stgf")
                            nc.vector.tensor_copy(stgf[:, :], psT[:, :])
                            Sp = sgen_p.tile([P, P], F32, name="Sp", tag="Sp")
                            nc.vector.tensor_tensor(
                                out=Sp[:, :].rearrange("p (t j) -> p t j", j=P),
                                in0=spool_sb[:, blk:blk + 1, None
                                             ].to_broadcast([P, 1, P]),
                                in1=iotaf_sb[:, None, :].to_broadcast([P, 1, P]),
                                op=iseq)
                            nc.tensor.matmul(
                                psPool[:, :], lhsT=Sp[:, :], rhs=stgf[:, :],
                                start=(blk == 0), stop=(blk == NBLK - 1))

                if debug_outs and li == 0:
                    nc.sync.dma_start(dbg_h0[:, :], hT_sb[:, :])
                if li < 2:
                    nc.gpsimd.collective_compute(
                        "AllGather", mybir.AluOpType.bypass,
                        replica_groups=groups,
                        ins=[tabin.opt()], outs=[tabouts[li].opt()],
                    )
                if debug_outs and li == 0:
                    nc.gpsimd.dma_start(dbg_tab0[:, :], tabouts[0][:, :])

            # ---- pooling: scatter partial sums, AllReduce ----
            pooled_sb = head_p.tile([P, HID], F32, name="pooled_sb")
            nc.scalar.copy(pooled_sb[:, :], psPool[:, :])
            lneps = head_p.tile([P, 1], F32, name="lneps")
            nc.vector.memset(lneps[:], LN_EPS)
            zt = head_p.tile([P, HID], F32, name="zt")
            nc.vector.memset(zt[:], 0.0)
            for j in range(NGT):
                nc.sync.dma_start(pools_t[j * P:(j + 1) * P, :], zt[:, :])
            nc.gpsimd.indirect_dma_start(
                out=pools_t[:, :],
                out_offset=IndirectOffsetOnAxis(ap=gidx_sb[:, :1], axis=0),
                in_=pooled_sb[:, :],
                in_offset=None,
            )
            nc.gpsimd.collective_compute(
                "AllReduce", add, replica_groups=groups,
                ins=[pools_t.opt()], outs=[poolr_t.opt()],
            )
            if debug_outs:
                nc.gpsimd.dma_start(dbg_pool[:, :], poolr_t[:, :])

            # ---- head (replicated on all cores) ----
            pts = []
            for gt in range(NGT):
                pt = pt_p.tile([P, HID], F32, name=f"pt{gt}", tag=f"pt{gt}")
                nc.sync.dma_start(pt[:, :], poolr_t[gt * P:(gt + 1) * P, :])
                nc.vector.tensor_scalar_mul(pt[:, :], pt[:, :],
                                            invc_sb[:, gt:gt + 1])
                psTT = psT_p.tile([HID, P], F32, name="psTT", tag="psT")
                nc.tensor.transpose(psTT[:, :], pt[:, :], ident_sb[:, :])
                nc.vector.tensor_copy(pooledT_sb[:, gt * P:(gt + 1) * P],
                                      psTT[:, :])
                pts.append(pt)

            for gt in range(NGT):
                gsl = slice(gt * P, (gt + 1) * P)
                psZ = psZ_p.tile([P, HID], F32, name="psZh", tag="psZ")
                nc.tensor.matmul(psZ[:, :], lhsT=pooledT_sb[:, gsl],
                                 rhs=fc1w_sb[:, :], start=True, stop=True)
                z1 = head_p.tile([P, HID], F32, name="z1", tag="z1")
                nc.vector.tensor_tensor(z1[:, :], psZ[:, :], fc1br_sb[:, :],
                                        op=add)
                mu = head_p.tile([P, 1], F32, name="mu", tag="mu")
                nc.vector.tensor_reduce(mu[:, :], z1[:, :],
                                        axis=mybir.AxisListType.X, op=add)
                nc.vector.tensor_scalar_mul(mu[:, :], mu[:, :], 1.0 / HID)
                xc = head_p.tile([P, HID], F32, name="xc", tag="xc")
                nc.vector.tensor_scalar(xc[:, :], z1[:, :], mu[:, :1], None,
                                        op0=mybir.AluOpType.subtract)
                sq = head_p.tile([P, HID], F32, name="sq", tag="sq")
                nc.vector.tensor_tensor(sq[:, :], xc[:, :], xc[:, :], op=mult)
                var = head_p.tile([P, 1], F32, name="var", tag="var")
                nc.vector.tensor_reduce(var[:, :], sq[:, :],
                                        axis=mybir.AxisListType.X, op=add)
                nc.vector.tensor_scalar_mul(var[:, :], var[:, :], 1.0 / HID)
                std = head_p.tile([P, 1], F32, name="std", tag="std")
                nc.scalar.activation(std[:, :], var[:, :],
                                     mybir.ActivationFunctionType.Sqrt,
                                     bias=lneps[:, :1], scale=1.0)
                rstd = head_p.tile([P, 1], F32, name="rstd", tag="rstd")
                nc.vector.reciprocal(rstd[:, :], std[:, :])
                nc.vector.tensor_scalar(xc[:, :], xc[:, :], rstd[:, :1], None,
                                        op0=mult)
                nc.vector.tensor_tensor(xc[:, :], xc[:, :], lngr_sb[:, :],
                                        op=mult)
                nc.vector.tensor_tensor(xc[:, :], xc[:, :], lnbr_sb[:, :],
                                        op=add)
                gl = head_p.tile([P, HID], F32, name="gl", tag="gl")
                nc.scalar.activation(gl[:, :], xc[:, :], GELU_FN)
                nc.vector.tensor_tensor(gl[:, :], gl[:, :], pts[gt][:, :],
                                        op=add)
                psR = psT_p.tile([HID, P], F32, name="psR", tag="psT")
                nc.tensor.transpose(psR[:, :], gl[:, :], ident_sb[:, :])
                resT = head_p.tile([HID, P], F32, name="resT", tag="resT")
                nc.vector.tensor_copy(resT[:, :], psR[:, :])
                psO = psZ_p.tile([P, LAT], F32, name="psO", tag="psZ")
                nc.tensor.matmul(psO[:, :], lhsT=resT[:, :], rhs=fc2w_sb[:, :],
                                 start=True, stop=True)
                ob = head_p.tile([P, LAT], F32, name="ob", tag="ob")
                nc.vector.tensor_tensor(ob[:, :], psO[:, :], fc2br_sb[:, :],
                                        op=add)
                nc.sync.dma_start(out_d[gsl, :], ob[:, :])

    nc.compile()
    return nc


_CACHE = {}


def _get_program(inputs):
    struct, in_maps = _prepare(inputs)
    key = (struct["N"], struct["E"], struct["G"], struct["IN_DIM"],
           struct["HID"], struct["LAT"], struct["NT"], tuple(struct["eps"]))
    if key not in _CACHE:
        _CACHE[key] = _build(struct)
    return _CACHE[key], struct, in_maps


def kernel(**inputs) -> np.ndarray:
    nc, struct, in_maps = _get_program(inputs)
    res = bass_utils.run_bass_kernel_spmd(nc, in_maps,
                                          core_ids=list(range(NCORES)))
    out = res.results[0]["out"]
    return np.ascontiguousarray(out[:struct["G"]]).astype(np.float32)
